# revision 9
# baseline (speedup 1.0000x reference)
"""Trainium2 Bass kernel: masked multi-head attention (B=2, S=2048, D=512, H=8).

Sharding: (head-pair x query-half) across 8 cores (core = hp*2 + qh).
Every core processes BOTH batches for its 2 heads and its 1024 queries:
unit u0 = batch 0 (nch0 key chunks), unit u1 = batch 1 (nch1 chunks).
This balances work across cores when valid_lens differ per batch (each
core gets nch0+nch1 chunks instead of 2*max(nch)).

Per (unit, head): scores^T -> exp (key-padding mask folded in as a
per-partition bias on the ScalarE) -> attn@v with a ones-column in V
(softmax denominator for free, row 64 of the PSUM result) -> per-head
normalize: denominator row is broadcast across 64 partitions with a
tiny ones-matmul on the PE (float32r, 1 cycle/col), reciprocal +
multiply on the VectorE reading the attention result directly from
PSUM -> per-(unit, qtile) partial out-projection -> store.  The 4
per-(batch, q-half) partials are summed on the host (the "all-reduce"),
then bias bo is added.

All tile pools are co-resident (PSUM budget: 2+4+2 = 8 banks) so no
phase barrier ever serializes the pipeline; the Tile scheduler overlaps
projections, attention, normalize and stores by data dependency.

The kernel specializes on (nch0, nch1) = ceil(valid_len_b/128): key
positions >= valid_len get exp(-30000) == 0 weight, so chunks past the
bound are skipped.  Derived from runtime inputs -> correct for any
valid_lens.
"""

import math
import os
import sys

import numpy as np

for _p in ("/opt/trn_rl_repo",):
    if os.path.isdir(_p) and _p not in sys.path:
        sys.path.insert(0, _p)

import ml_dtypes

D_MODEL = 512
NUM_HEADS = 8
HEAD_DIM = 64
N_CORES = 8
LOCAL_F = 128            # features per core = 2 heads * 64
VAUG = 2 * HEAD_DIM + 2  # 130: [v_h0 (64) | ones | v_h1 (64) | ones]
MASK_NEG = -30000.0

DT_NAME = os.environ.get("ATTN_KERNEL_DT", "bfloat16")
TRACE = False

last_results = None  # BassKernelResults of the most recent run (for test.py)

_PROG_CACHE = {}


def _np_dt(name):
    return ml_dtypes.bfloat16 if name == "bfloat16" else np.float32


def _build(nch0: int, nch1: int, seq: int, dt_name: str,
           qk_bias: bool, v_bias: bool):
    from contextlib import ExitStack

    import concourse.bass as bass  # noqa: F401
    import concourse.mybir as mybir
    import concourse.tile as tile
    from concourse import bacc

    DT = getattr(mybir.dt, dt_name)
    F32 = mybir.dt.float32
    F32R = mybir.dt.float32r
    EXP = mybir.ActivationFunctionType.Exp
    QU = seq // 2            # queries per unit
    assert QU % 512 == 0
    NQT = QU // 512          # 512-wide query tiles per unit
    sk0, sk1 = nch0 * 128, nch1 * 128
    SKT = sk0 + sk1
    NCH = nch0 + nch1
    units = [(0, nch0, 0), (1, nch1, nch0)]  # (unit, nch, chunk offset)

    nc = bacc.Bacc("TRN2", target_bir_lowering=False, debug=False,
                   num_devices=N_CORES)

    def din(name, shape, dt=DT):
        return nc.dram_tensor(name, shape, dt, kind="ExternalInput").ap()

    xTq = din("xTq", [D_MODEL, seq])
    xTk = din("xTk", [D_MODEL, SKT])
    xTv = din("xTv", [D_MODEL, SKT])
    WQKV = 2 * LOCAL_F + VAUG        # [wqT | wkT | wvT_aug] column blocks
    wqkv = din("wqkv", [D_MODEL, WQKV])
    woT = din("woT", [LOCAL_F, D_MODEL])
    # f32 smalls: [bq | bk | bv_aug(VAUG) | maskb(NCH)]
    NSM = 2 + VAUG + NCH
    smalls_d = din("smalls", [128, NSM], F32)
    out_d = nc.dram_tensor("out", [D_MODEL, seq], DT,
                           kind="ExternalOutput").ap()

    with tile.TileContext(nc) as tc, ExitStack() as ctx:
        const = ctx.enter_context(tc.tile_pool(name="const", bufs=1))

        # ---- stage inputs into SBUF ----
        # small/weight loads on the Pool (gpsimd) queue: 25ns issue each,
        # leaves the sync HWDGE queue free for the big input tensors.
        sm_sb = const.tile([128, NSM], F32, tag="sm")
        nc.gpsimd.dma_start(out=sm_sb, in_=smalls_d)
        wqkv_sb = const.tile([128, 4, WQKV], DT, tag="wqkv")
        wqkv_r = wqkv.rearrange("(c p) f -> p c f", p=128)
        # wk block first — the k projection is the first consumer
        nc.gpsimd.dma_start(out=wqkv_sb[:, :, LOCAL_F:2 * LOCAL_F],
                            in_=wqkv_r[:, :, LOCAL_F:2 * LOCAL_F])
        nc.gpsimd.dma_start(out=wqkv_sb[:, :, 0:LOCAL_F],
                            in_=wqkv_r[:, :, 0:LOCAL_F])
        nc.gpsimd.dma_start(out=wqkv_sb[:, :, 2 * LOCAL_F:],
                            in_=wqkv_r[:, :, 2 * LOCAL_F:])

        # big inputs: k first (first consumer), then q of unit 0, v, q of u1
        xk_sb = const.tile([128, 4, SKT], DT, tag="xk")
        for c in range(4):
            nc.sync.dma_start(out=xk_sb[:, c, :],
                              in_=xTk[c * 128:(c + 1) * 128, :])
        xq_sb = const.tile([128, 4, seq], DT, tag="xq")
        for c in range(4):
            nc.sync.dma_start(out=xq_sb[:, c, 0:QU],
                              in_=xTq[c * 128:(c + 1) * 128, 0:QU])
        xv_sb = const.tile([128, 4, SKT], DT, tag="xv")
        for c in range(4):
            nc.scalar.dma_start(out=xv_sb[:, c, :],
                                in_=xTv[c * 128:(c + 1) * 128, :])
        for c in range(4):
            nc.sync.dma_start(out=xq_sb[:, c, QU:seq],
                              in_=xTq[c * 128:(c + 1) * 128, QU:seq])
        wo_sb = const.tile([LOCAL_F, D_MODEL], DT, tag="wo")
        nc.gpsimd.dma_start(out=wo_sb, in_=woT)

        bq_sb = sm_sb[:, 0:1]
        bk_sb = sm_sb[:, 1:2]
        bv_sb = sm_sb[:, 2:2 + VAUG]
        mb_sb = sm_sb[:, 2 + VAUG:2 + VAUG + NCH]
        wq_of, wk_of, wv_of = 0, LOCAL_F, 2 * LOCAL_F

        # ---- persistent SBUF operands ----
        qT = const.tile([LOCAL_F, seq], DT, tag="qT")
        kT = const.tile([LOCAL_F, SKT], DT, tag="kT")
        vaug = const.tile([128, NCH, VAUG], DT, tag="vaug")
        cn = const.tile([LOCAL_F, seq], DT, tag="cn")
        den2 = const.tile([1, 2, seq], DT, tag="den2")
        sel1 = const.tile([1, HEAD_DIM], DT, tag="sel1")
        nc.vector.memset(sel1, 1.0)
        for _oc in (64, VAUG - 1):
            _ones = vaug[:, :, _oc:_oc + 1]
            if dt_name == "float32r":  # memset can't encode f32r
                _ones = _ones.bitcast(F32)
            nc.vector.memset(_ones, 1.0)

        with (
            tc.tile_pool(name="swork", bufs=2, space="PSUM") as swork,
            tc.tile_pool(name="scps", bufs=2, space="PSUM") as scps,
            tc.tile_pool(name="otps", bufs=2, space="PSUM") as otps,
            tc.tile_pool(name="expp", bufs=7) as expp,
            tc.tile_pool(name="obp", bufs=4) as obp,
            tc.tile_pool(name="rcsp", bufs=3) as rcsp,
        ):
            # PE warm-up: ramps the HAM clock while staging DMAs land
            warm = const.tile([128, 512], DT, tag="warm")
            nc.vector.memset(warm, 0.0)
            for i in range(4):
                wps = swork.tile([128, 512], F32, tag="sw", name=f"warm{i}")
                nc.tensor.matmul(wps, lhsT=warm[:, 0:128], rhs=warm,
                                 start=True, stop=True)

            # ---- k projection ----
            # u0 (N=384) via swork; u1 (N=640) via a scps tile
            kp0 = swork.tile([128, 512], F32, tag="sw", name="kp0")
            for dc in range(4):
                nc.tensor.matmul(kp0[:, 0:sk0],
                                 lhsT=wqkv_sb[:, dc, wk_of:wk_of + LOCAL_F],
                                 rhs=xk_sb[:, dc, 0:sk0],
                                 start=(dc == 0), stop=(dc == 3))
            nc.scalar.copy(out=kT[:, 0:sk0], in_=kp0[:, 0:sk0])
            if qk_bias:
                nc.vector.tensor_scalar_add(out=kT[:, 0:sk0],
                                            in0=kT[:, 0:sk0], scalar1=bk_sb)
            kp1 = scps.tile([128, 1024], F32, tag="sc", name="kp1")
            for j0 in range(0, sk1, 512):
                w = min(512, sk1 - j0)
                for dc in range(4):
                    nc.tensor.matmul(
                        kp1[:, j0:j0 + w],
                        lhsT=wqkv_sb[:, dc, wk_of:wk_of + LOCAL_F],
                        rhs=xk_sb[:, dc, sk0 + j0:sk0 + j0 + w],
                        start=(dc == 0), stop=(dc == 3))
            nc.scalar.copy(out=kT[:, sk0:SKT], in_=kp1[:, 0:sk1])
            if qk_bias:
                nc.vector.tensor_scalar_add(out=kT[:, sk0:SKT],
                                            in0=kT[:, sk0:SKT], scalar1=bk_sb)

            def q_proj(u):
                q0 = u * QU
                for half in range(QU // 1024):
                    qp = scps.tile([128, 1024], F32, tag="sc",
                                   name=f"qp{u}{half}")
                    for t in range(2):
                        for dc in range(4):
                            nc.tensor.matmul(
                                qp[:, t * 512:(t + 1) * 512],
                                lhsT=wqkv_sb[:, dc, wq_of:wq_of + LOCAL_F],
                                rhs=xq_sb[:, dc, q0 + half * 1024 + t * 512:
                                          q0 + half * 1024 + (t + 1) * 512],
                                start=(dc == 0), stop=(dc == 3))
                    dst = qT[:, q0 + half * 1024:q0 + (half + 1) * 1024]
                    nc.vector.tensor_copy(out=dst, in_=qp)
                    if qk_bias:
                        nc.vector.tensor_scalar_add(out=dst, in0=dst,
                                                    scalar1=bq_sb)

            def v_proj(u):
                _, nch, coff = units[u]
                for c in range(nch):
                    vp = swork.tile([128, 512], F32, tag="sw",
                                    name=f"vp{u}{c}")
                    for dc in range(4):
                        nc.tensor.matmul(
                            vp[:, 0:VAUG],
                            lhsT=xv_sb[:, dc,
                                       (coff + c) * 128:(coff + c + 1) * 128],
                            rhs=wqkv_sb[:, dc, wv_of:wv_of + VAUG],
                            start=(dc == 0), stop=(dc == 3))
                    gc = coff + c
                    nc.scalar.copy(out=vaug[:, gc, 0:64], in_=vp[:, 0:64])
                    nc.scalar.copy(out=vaug[:, gc, 65:129], in_=vp[:, 65:129])
                    if v_bias:
                        nc.vector.tensor_add(out=vaug[:, gc, 0:64],
                                             in0=vaug[:, gc, 0:64],
                                             in1=bv_sb[:, 0:64])
                        nc.vector.tensor_add(out=vaug[:, gc, 65:129],
                                             in0=vaug[:, gc, 65:129],
                                             in1=bv_sb[:, 65:129])

            def attention_unit(u, cast_engines):
                _, nch, coff = units[u]
                ucol = u * QU
                for h in range(2):
                    # scores + exp per chunk (1024-wide ACT ops), then
                    # attn@v per 512-wide q tile accumulating over chunks
                    exs = []
                    for c in range(nch):
                        sc = scps.tile([128, 1024], F32, tag="sc",
                                       name=f"sc{u}{h}{c}")
                        for t in range(NQT):
                            nc.tensor.matmul(
                                sc[:, t * 512:(t + 1) * 512],
                                lhsT=kT[h * 64:(h + 1) * 64,
                                        coff * 128 + c * 128:
                                        coff * 128 + (c + 1) * 128],
                                rhs=qT[h * 64:(h + 1) * 64,
                                       ucol + t * 512:ucol + (t + 1) * 512],
                                start=True, stop=True)
                        ex = expp.tile([128, 1024], DT, tag="ex",
                                       name=f"ex{u}{h}{c}")
                        nc.scalar.activation(
                            out=ex, in_=sc, func=EXP,
                            bias=mb_sb[:, coff + c:coff + c + 1],
                            scale=1.0 / math.sqrt(HEAD_DIM))
                        exs.append(ex)
                    for t in range(NQT):
                        oT = otps.tile([65, 512], F32, tag="oT",
                                       name=f"oT{u}{h}{t}")
                        for c in range(nch):
                            nc.tensor.matmul(
                                oT,
                                lhsT=vaug[:, coff + c, h * 65:(h + 1) * 65],
                                rhs=exs[c][:, t * 512:(t + 1) * 512],
                                start=(c == 0), stop=(c == nch - 1))
                        csl = slice(ucol + t * 512, ucol + (t + 1) * 512)
                        # denominator row -> SBUF bf16 (DVE), broadcast to
                        # the head's 64 partitions with a 1x64 ones-matmul
                        # (PE), reciprocal into SBUF, normalize (DVE; every
                        # op reads at most one PSUM operand)
                        nc.vector.tensor_copy(out=den2[0:1, h, csl],
                                              in_=oT[64:65, :])
                        bc = swork.tile([64, 512], F32, tag="sw",
                                        name=f"bc{u}{h}{t}")
                        nc.tensor.matmul(bc, lhsT=sel1,
                                         rhs=den2[0:1, h, csl],
                                         start=True, stop=True)
                        rcs = rcsp.tile([64, 512], F32, tag="rcs",
                                        name=f"rcs{u}{h}{t}")
                        nc.vector.reciprocal_approx_fast(out=rcs, in_=bc)
                        nc.vector.tensor_mul(
                            out=cn[h * 64:(h + 1) * 64, csl],
                            in0=oT[0:64, :], in1=rcs)
                # out-projection for this unit (needs both heads' cn)
                for t in range(NQT):
                    csl = slice(ucol + t * 512, ucol + (t + 1) * 512)
                    for odc in range(4):
                        fp = swork.tile([128, 512], F32, tag="sw",
                                        name=f"fp{u}{t}{odc}")
                        nc.tensor.matmul(
                            fp, lhsT=wo_sb[:, odc * 128:(odc + 1) * 128],
                            rhs=cn[:, csl], start=True, stop=True)
                        ob = obp.tile([128, 512], DT, tag="ob",
                                      name=f"ob{u}{t}{odc}")
                        eng = cast_engines[(t * 4 + odc) % len(cast_engines)]
                        if eng == "v":
                            nc.vector.tensor_copy(out=ob, in_=fp)
                        else:
                            nc.scalar.copy(out=ob, in_=fp)
                        nc.gpsimd.dma_start(
                            out=out_d[odc * 128:(odc + 1) * 128, csl],
                            in_=ob)

            q_proj(0)
            v_proj(0)
            attention_unit(0, cast_engines=("v", "s"))
            q_proj(1)
            v_proj(1)
            # exp stream is done by the tail of unit 1 -> ScalarE helps cast
            attention_unit(1, cast_engines=("v", "s"))

    nc.compile()
    return nc


def kernel(queries, keys, values, valid_lens, Wq, bq, Wk, bk, Wv, bv, Wo, bo):
    global last_results
    queries = np.asarray(queries, dtype=np.float32)
    keys = np.asarray(keys, dtype=np.float32)
    values = np.asarray(values, dtype=np.float32)
    valid_lens = np.asarray(valid_lens).astype(np.int64)
    Wq = np.asarray(Wq, dtype=np.float32)
    Wk = np.asarray(Wk, dtype=np.float32)
    Wv = np.asarray(Wv, dtype=np.float32)
    Wo = np.asarray(Wo, dtype=np.float32)
    bq = np.asarray(bq, dtype=np.float32)
    bk = np.asarray(bk, dtype=np.float32)
    bv = np.asarray(bv, dtype=np.float32)
    bo = np.asarray(bo, dtype=np.float32)

    B, S, D = queries.shape
    assert (B, D) == (2, D_MODEL) and S % 2048 == 0
    QU = S // 2

    Ls = [int(min(max(int(valid_lens[b]), 1), S)) for b in range(B)]
    nchs = [(L + 127) // 128 for L in Ls]
    nch0, nch1 = nchs
    sk0, sk1 = nch0 * 128, nch1 * 128
    NCH = nch0 + nch1

    npdt = _np_dt(DT_NAME)
    qk_bias = bool(np.any(bq) or np.any(bk))
    v_bias = bool(np.any(bv))
    key = (nch0, nch1, S, DT_NAME, qk_bias, v_bias)
    if key not in _PROG_CACHE:
        _PROG_CACHE[key] = _build(nch0, nch1, S, DT_NAME, qk_bias, v_bias)
    nc = _PROG_CACHE[key]

    masks = []
    for b in range(B):
        skb = nchs[b] * 128
        m = np.where(np.arange(skb) < Ls[b], 0.0, MASK_NEG).astype(np.float32)
        masks.append(m.reshape(nchs[b], 128).T)

    in_maps = []
    for core in range(N_CORES):
        hp, qh = divmod(core, 2)
        fs = hp * LOCAL_F
        wvT_aug = np.zeros((D, VAUG), np.float32)
        wvT_aug[:, 0:64] = Wv[fs:fs + 64, :].T
        wvT_aug[:, 65:129] = Wv[fs + 64:fs + 128, :].T
        bv_aug = np.zeros((VAUG,), np.float32)
        bv_aug[0:64] = bv[fs:fs + 64]
        bv_aug[64] = 1.0
        bv_aug[65:129] = bv[fs + 64:fs + 128]
        bv_aug[129] = 1.0
        wqkv = np.concatenate(
            [Wq[fs:fs + 128, :].T, Wk[fs:fs + 128, :].T, wvT_aug], axis=1)
        smalls = np.empty((128, 2 + VAUG + NCH), np.float32)
        smalls[:, 0] = bq[fs:fs + 128]
        smalls[:, 1] = bk[fs:fs + 128]
        smalls[:, 2:2 + VAUG] = bv_aug
        smalls[:, 2 + VAUG:2 + VAUG + nch0] = masks[0]
        smalls[:, 2 + VAUG + nch0:] = masks[1]
        qsl = slice(qh * QU, (qh + 1) * QU)
        xTq = np.concatenate(
            [queries[0, qsl].T, queries[1, qsl].T], axis=1)
        xTk = np.concatenate([keys[0, :sk0].T, keys[1, :sk1].T], axis=1)
        xTv = np.concatenate([values[0, :sk0].T, values[1, :sk1].T], axis=1)
        in_maps.append({
            "xTq": np.ascontiguousarray(xTq).astype(npdt),
            "xTk": np.ascontiguousarray(xTk).astype(npdt),
            "xTv": np.ascontiguousarray(xTv).astype(npdt),
            "wqkv": np.ascontiguousarray(wqkv).astype(npdt),
            "woT": np.ascontiguousarray(Wo[:, fs:fs + 128].T).astype(npdt),
            "smalls": smalls,
        })

    from concourse.bass_utils import run_bass_kernel_spmd
    res = run_bass_kernel_spmd(nc, in_maps, list(range(N_CORES)), trace=TRACE)
    last_results = res
    outs = [r["out"] for r in res.results]

    final = np.empty((B, S, D), np.float32)
    for b in range(B):
        for qh in range(2):
            acc = sum(outs[hp * 2 + qh][:, b * QU:(b + 1) * QU]
                      .astype(np.float32) for hp in range(4))
            final[b, qh * QU:(qh + 1) * QU] = acc.T + bo
        if int(valid_lens[b]) == 0:
            # uniform attention over all S positions (reference semantics
            # when every key is masked: softmax of a constant row)
            row = (values[b].mean(0) @ Wv.T + bv) @ Wo.T + bo
            final[b] = np.broadcast_to(row, (S, D))
    return final


# revision 15
# speedup vs baseline: 1.0621x; 1.0621x over previous
"""Trainium2 Bass kernel: masked multi-head attention (B=2, S=2048, D=512, H=8).

Sharding: (head-pair x query-half) across 8 cores (core = hp*2 + qh).
Every core processes BOTH batches for its 2 heads and its 1024 queries:
unit u0 = batch 0 (nch0 key chunks), unit u1 = batch 1 (nch1 chunks),
which balances work across cores when valid_lens differ per batch.

Inputs are packed chunk-major on the host so each projection can start
as soon as its own DMA lands: keys/values as [128, chunk, dc, 128],
queries as [128, qtile, dc, 512].  The attention inner loop interleaves
scores(c) / exp(c) / attn@v(c-1) so the in-order PE queue never blocks
on the ScalarE, keeping the tensor engine at full p-state.

Softmax: the mask is a per-partition bias on the exp activation; a ones
column in the augmented V gives the denominator as row 64 of the attn@v
PSUM tile.  Normalize does reciprocal straight off that PSUM row into
SBUF (one DVE op), broadcasts it across 64 partitions via a DRAM bounce
(DMA only — broadcast access patterns are legal on DRAM APs), then one
multiply per head reading the PSUM numerators directly.

The per-core partial out-projection [512, 1024] per batch is stored and
summed on the host (the "all-reduce"), then bias bo is added.

The kernel specializes on (nch0, nch1) = ceil(valid_len_b/128): key
positions >= valid_len get exp(-30000) == 0 weight, so chunks past the
bound are skipped.  Derived from runtime inputs -> correct for any
valid_lens.
"""

import math
import os
import sys

import numpy as np

for _p in ("/opt/trn_rl_repo",):
    if os.path.isdir(_p) and _p not in sys.path:
        sys.path.insert(0, _p)

import ml_dtypes

D_MODEL = 512
NUM_HEADS = 8
HEAD_DIM = 64
N_CORES = 8
LOCAL_F = 128            # features per core = 2 heads * 64
MASK_NEG = -30000.0

DT_NAME = os.environ.get("ATTN_KERNEL_DT", "bfloat16")
TRACE = False

last_results = None  # BassKernelResults of the most recent run (for test.py)

_PROG_CACHE = {}


def _np_dt(name):
    return ml_dtypes.bfloat16 if name == "bfloat16" else np.float32


def _build(nch0: int, nch1: int, seq: int, dt_name: str,
           qk_bias: bool, v_bias: bool):
    from contextlib import ExitStack

    import concourse.bass as bass  # noqa: F401
    import concourse.mybir as mybir
    import concourse.tile as tile
    from concourse import bacc

    DT = getattr(mybir.dt, dt_name)
    F32 = mybir.dt.float32
    EXP = mybir.ActivationFunctionType.Exp
    QU = seq // 2            # queries per unit
    assert QU % 512 == 0
    NQT = QU // 512          # 512-wide query tiles per unit
    NT = 2 * NQT             # query tiles total
    sk0, sk1 = nch0 * 128, nch1 * 128
    NCH = nch0 + nch1
    units = [(0, nch0, 0), (1, nch1, nch0)]  # (unit, nch, chunk offset)

    nc = bacc.Bacc("TRN2", target_bir_lowering=False, debug=False,
                   num_devices=N_CORES)

    def din(name, shape, dt=DT):
        return nc.dram_tensor(name, shape, dt, kind="ExternalInput").ap()

    # chunk-/tile-major packed inputs (see host packing in kernel())
    xq_d = din("xq", [128, NT, 4, 512])
    xk_d = din("xk", [128, NCH, 4, 128])
    xv_d = din("xv", [128, NCH, 4, 128])
    # [wk | wq | wv] column blocks, each 128 wide
    wqkv = din("wqkv", [D_MODEL, 3 * LOCAL_F])
    woT = din("woT", [LOCAL_F, D_MODEL])
    # f32 smalls: [bq | bk | bv | maskb(NCH)]
    NSM = 2 + LOCAL_F + NCH
    smalls_d = din("smalls", [128, NSM], F32)
    out_d = nc.dram_tensor("out", [D_MODEL, seq], DT,
                           kind="ExternalOutput").ap()

    with tile.TileContext(nc) as tc, ExitStack() as ctx:
        const = ctx.enter_context(tc.tile_pool(name="const", bufs=1))

        # ---- stage inputs into SBUF ----
        # weights/smalls on the Pool queue (cheap issue), k+q on the sync
        # HWDGE queue in consumption order, v on the scalar queue.
        sm_sb = const.tile([128, NSM], F32, tag="sm")
        nc.gpsimd.dma_start(out=sm_sb, in_=smalls_d)
        wqkv_sb = const.tile([128, 4, 3 * LOCAL_F], DT, tag="wqkv")
        wqkv_r = wqkv.rearrange("(c p) f -> p c f", p=128)
        nc.gpsimd.dma_start(out=wqkv_sb[:, :, 0:2 * LOCAL_F],
                            in_=wqkv_r[:, :, 0:2 * LOCAL_F])
        nc.gpsimd.dma_start(out=wqkv_sb[:, :, 2 * LOCAL_F:],
                            in_=wqkv_r[:, :, 2 * LOCAL_F:])

        xk_sb = const.tile([128, NCH, 4, 128], DT, tag="xk")
        xq_sb = const.tile([128, NT, 4, 512], DT, tag="xq")
        xv_sb = const.tile([128, NCH, 4, 128], DT, tag="xv")
        nc.sync.dma_start(out=xk_sb[:, 0:nch0], in_=xk_d[:, 0:nch0])
        nc.sync.dma_start(out=xq_sb[:, 0], in_=xq_d[:, 0])
        nc.sync.dma_start(out=xk_sb[:, nch0:], in_=xk_d[:, nch0:])
        nc.scalar.dma_start(out=xv_sb[:, 0:nch0], in_=xv_d[:, 0:nch0])
        for t in range(1, NT):
            nc.sync.dma_start(out=xq_sb[:, t], in_=xq_d[:, t])
        nc.scalar.dma_start(out=xv_sb[:, nch0:], in_=xv_d[:, nch0:])
        wo_sb = const.tile([LOCAL_F, D_MODEL], DT, tag="wo")
        nc.gpsimd.dma_start(out=wo_sb, in_=woT)

        bq_sb = sm_sb[:, 0:1]
        bk_sb = sm_sb[:, 1:2]
        bv_sb = sm_sb[:, 2:2 + LOCAL_F]
        mb_sb = sm_sb[:, 2 + LOCAL_F:2 + LOCAL_F + NCH]
        wk_of, wq_of, wv_of = 0, LOCAL_F, 2 * LOCAL_F

        # ---- persistent SBUF operands ----
        qT = const.tile([LOCAL_F, seq], DT, tag="qT")
        kT = const.tile([LOCAL_F, NCH * 128], DT, tag="kT")
        # per chunk: [:, c, h, 0:64] = 1.0, [:, c, h, 64:128] = V_h — the
        # 64 ones columns replicate the softmax denominator into PSUM rows
        # 0..63 of the attn@v result (a free partition-broadcast: matmul
        # cost only depends on the moving size N, not on M), where the
        # custom reciprocal op can read it at base partition 0
        vaug = const.tile([128, NCH, 2, LOCAL_F], DT, tag="vaug")
        cn = const.tile([LOCAL_F, seq], DT, tag="cn")
        nc.vector.memset(vaug[:, :, :, 0:HEAD_DIM], 1.0)

        with (
            tc.tile_pool(name="swork", bufs=2, space="PSUM") as swork,
            tc.tile_pool(name="scps", bufs=2, space="PSUM") as scps,
            tc.tile_pool(name="otps", bufs=2, space="PSUM") as otps,
            tc.tile_pool(name="expp", bufs=7) as expp,
            tc.tile_pool(name="obp", bufs=4) as obp,
            tc.tile_pool(name="rbp", bufs=4) as rbp,
        ):
            # PE warm-up: ramps the HAM clock while the first DMAs land
            warm = const.tile([128, 512], DT, tag="warm")
            nc.vector.memset(warm, 0.0)
            for i in range(2):
                wps = swork.tile([128, 512], F32, tag="sw", name=f"warm{i}")
                nc.tensor.matmul(wps, lhsT=warm[:, 0:128], rhs=warm,
                                 start=True, stop=True)

            def k_proj_chunk(c, eng):
                kp = swork.tile([128, 512], F32, tag="sw", name=f"kp{c}")
                for dc in range(4):
                    nc.tensor.matmul(kp[:, 0:128],
                                     lhsT=wqkv_sb[:, dc, wk_of:wk_of + 128],
                                     rhs=xk_sb[:, c, dc, :],
                                     start=(dc == 0), stop=(dc == 3))
                dst = kT[:, c * 128:(c + 1) * 128]
                (nc.scalar.copy(out=dst, in_=kp[:, 0:128]) if eng == "s"
                 else nc.vector.tensor_copy(out=dst, in_=kp[:, 0:128]))
                if qk_bias:
                    nc.vector.tensor_scalar_add(out=dst, in0=dst,
                                                scalar1=bk_sb)

            def q_proj_tile(t, eng):
                qp = swork.tile([128, 512], F32, tag="sw", name=f"qp{t}")
                for dc in range(4):
                    nc.tensor.matmul(qp,
                                     lhsT=wqkv_sb[:, dc, wq_of:wq_of + 128],
                                     rhs=xq_sb[:, t, dc, :],
                                     start=(dc == 0), stop=(dc == 3))
                dst = qT[:, t * 512:(t + 1) * 512]
                (nc.scalar.copy(out=dst, in_=qp) if eng == "s"
                 else nc.vector.tensor_copy(out=dst, in_=qp))
                if qk_bias:
                    nc.vector.tensor_scalar_add(out=dst, in0=dst,
                                                scalar1=bq_sb)

            def v_proj_chunk(c, eng):
                vp = swork.tile([128, 512], F32, tag="sw", name=f"vp{c}")
                for dc in range(4):
                    nc.tensor.matmul(vp[:, 0:128],
                                     lhsT=xv_sb[:, c, dc, :],
                                     rhs=wqkv_sb[:, dc, wv_of:wv_of + 128],
                                     start=(dc == 0), stop=(dc == 3))
                if v_bias:
                    nc.vector.tensor_add(out=vp[:, 0:128], in0=vp[:, 0:128],
                                         in1=bv_sb)
                src = vp[:, 0:128].rearrange("p (h d) -> p h d", h=2)
                dst = vaug[:, c, :, HEAD_DIM:]
                (nc.scalar.copy(out=dst, in_=src) if eng == "s"
                 else nc.vector.tensor_copy(out=dst, in_=src))

            # projections for unit 0 first (attention starts on them),
            # then unit 1 interleaves under unit 0's attention stream
            for c in range(nch0):
                k_proj_chunk(c, "s")
            q_proj_tile(0, "s")
            for c in range(nch0):
                v_proj_chunk(c, "s")
            q_proj_tile(1, "v")

            def attention_head(u, h, pending, drain):
                """scores/exp/attn@v for one (unit, head). `pending` PE work
                (late projections / out-proj of the previous unit) is
                drained `drain` items per chunk iteration, hiding it in the
                exp-wait gaps of the stream."""
                _, nch, coff = units[u]
                ucol = u * QU
                exs = []
                oTs = {}

                def attn_v(c):
                    for t in range(NQT):
                        if c == 0:
                            oTs[t] = otps.tile([128, 512], F32, tag="oT",
                                               name=f"oT{u}{h}{t}")
                        nc.tensor.matmul(
                            oTs[t],
                            lhsT=vaug[:, coff + c, h, :],
                            rhs=exs[c][:, t * 512:(t + 1) * 512],
                            start=(c == 0), stop=(c == nch - 1))

                for c in range(nch):
                    sc = scps.tile([128, 1024], F32, tag="sc",
                                   name=f"sc{u}{h}{c}")
                    for t in range(NQT):
                        nc.tensor.matmul(
                            sc[:, t * 512:(t + 1) * 512],
                            lhsT=kT[h * 64:(h + 1) * 64,
                                    (coff + c) * 128:(coff + c + 1) * 128],
                            rhs=qT[h * 64:(h + 1) * 64,
                                   ucol + t * 512:ucol + (t + 1) * 512],
                            start=True, stop=True)
                    ex = expp.tile([128, 1024], DT, tag="ex",
                                   name=f"ex{u}{h}{c}")
                    nc.scalar.activation(
                        out=ex, in_=sc, func=EXP,
                        bias=mb_sb[:, coff + c:coff + c + 1],
                        scale=1.0 / math.sqrt(HEAD_DIM))
                    exs.append(ex)
                    if c > 0:
                        attn_v(c - 1)
                    for _ in range(drain):
                        if pending:
                            item = pending.pop(0)
                            if item is not None:
                                item()
                attn_v(nch - 1)

                for t in range(NQT):
                    csl = slice(ucol + t * 512, ucol + (t + 1) * 512)
                    oT = oTs[t]
                    rb = rbp.tile([64, 512], F32, tag="rb",
                                  name=f"rb{u}{h}{t}")
                    nc.vector.reciprocal_approx_fast(
                        out=rb, in_=oT[0:64, :])
                    nc.vector.tensor_mul(
                        out=cn[h * 64:(h + 1) * 64, csl],
                        in0=oT[64:128, :], in1=rb)

            def out_proj_item(u, t, eng, odcs=(0, 1, 2, 3)):
                def emit():
                    csl = slice(u * QU + t * 512, u * QU + (t + 1) * 512)
                    for odc in odcs:
                        fp = swork.tile([128, 512], F32, tag="sw",
                                        name=f"fp{u}{t}{odc}")
                        nc.tensor.matmul(
                            fp, lhsT=wo_sb[:, odc * 128:(odc + 1) * 128],
                            rhs=cn[:, csl], start=True, stop=True)
                        ob = obp.tile([128, 512], DT, tag="ob",
                                      name=f"ob{u}{t}{odc}")
                        (nc.scalar.copy(out=ob, in_=fp) if eng == "s"
                         else nc.vector.tensor_copy(out=ob, in_=fp))
                        nc.gpsimd.dma_start(
                            out=out_d[odc * 128:(odc + 1) * 128, csl],
                            in_=ob)
                return emit

            # unit-1 projections drain inside unit-0's attention stream
            # (k and q tiles first — consumed at the head boundaries);
            # unit-0's out-proj drains inside unit-1's stream after the
            # normalize bounce has round-tripped (None = idle slot)
            late0 = [lambda c=c: k_proj_chunk(c, "v") for c in range(nch0, NCH)]
            late0 += [lambda: q_proj_tile(2, "v"), lambda: q_proj_tile(3, "v")]
            late0 += [lambda c=c: v_proj_chunk(c, "v") for c in range(nch0, NCH)]
            attention_head(0, 0, late0, drain=2)
            attention_head(0, 1, late0, drain=2)
            late1 = [f for f in late0 if f is not None]
            late1 = late1 + [None,
                             out_proj_item(0, 0, "v", (0, 1)),
                             out_proj_item(0, 0, "v", (2, 3)),
                             out_proj_item(0, 1, "v", (0, 1)),
                             out_proj_item(0, 1, "v", (2, 3))]
            attention_head(1, 0, late1, drain=1)
            late2 = list(late1)
            attention_head(1, 1, late2, drain=1)
            for f in late2:
                if f is not None:
                    f()
            out_proj_item(1, 0, "s")()
            out_proj_item(1, 1, "v")()

    nc.compile()
    return nc


def kernel(queries, keys, values, valid_lens, Wq, bq, Wk, bk, Wv, bv, Wo, bo):
    global last_results
    queries = np.asarray(queries, dtype=np.float32)
    keys = np.asarray(keys, dtype=np.float32)
    values = np.asarray(values, dtype=np.float32)
    valid_lens = np.asarray(valid_lens).astype(np.int64)
    Wq = np.asarray(Wq, dtype=np.float32)
    Wk = np.asarray(Wk, dtype=np.float32)
    Wv = np.asarray(Wv, dtype=np.float32)
    Wo = np.asarray(Wo, dtype=np.float32)
    bq = np.asarray(bq, dtype=np.float32)
    bk = np.asarray(bk, dtype=np.float32)
    bv = np.asarray(bv, dtype=np.float32)
    bo = np.asarray(bo, dtype=np.float32)

    B, S, D = queries.shape
    assert (B, D) == (2, D_MODEL) and S % 2048 == 0
    QU = S // 2

    Ls = [int(min(max(int(valid_lens[b]), 1), S)) for b in range(B)]
    nchs = [(L + 127) // 128 for L in Ls]
    nch0, nch1 = nchs
    sk0, sk1 = nch0 * 128, nch1 * 128
    NCH = nch0 + nch1

    npdt = _np_dt(DT_NAME)
    qk_bias = bool(np.any(bq) or np.any(bk))
    v_bias = bool(np.any(bv))
    key = (nch0, nch1, S, DT_NAME, qk_bias, v_bias)
    if key not in _PROG_CACHE:
        _PROG_CACHE[key] = _build(nch0, nch1, S, DT_NAME, qk_bias, v_bias)
    nc = _PROG_CACHE[key]

    in_maps = _host_pack(queries, keys, values, nchs, Ls,
                         Wq, bq, Wk, bk, Wv, bv, Wo)

    from concourse.bass_utils import run_bass_kernel_spmd
    res = run_bass_kernel_spmd(nc, in_maps, list(range(N_CORES)), trace=TRACE)
    last_results = res
    outs = [r["out"] for r in res.results]

    final = np.empty((B, S, D), np.float32)
    for b in range(B):
        for qh in range(2):
            acc = sum(outs[hp * 2 + qh][:, b * QU:(b + 1) * QU]
                      .astype(np.float32) for hp in range(4))
            final[b, qh * QU:(qh + 1) * QU] = acc.T + bo
        if int(valid_lens[b]) == 0:
            # uniform attention over all S positions (reference semantics
            # when every key is masked: softmax of a constant row)
            row = (values[b].mean(0) @ Wv.T + bv) @ Wo.T + bo
            final[b] = np.broadcast_to(row, (S, D))
    return final


def _host_pack(queries, keys, values, nchs, Ls, Wq, bq, Wk, bk, Wv, bv, Wo):
    B, S, D = queries.shape
    QU = S // 2
    nch0, nch1 = nchs
    sk0, sk1 = nch0 * 128, nch1 * 128
    NCH = nch0 + nch1
    npdt = _np_dt(DT_NAME)

    masks = []
    for b in range(B):
        skb = nchs[b] * 128
        m = np.where(np.arange(skb) < Ls[b], 0.0, MASK_NEG).astype(np.float32)
        masks.append(m.reshape(nchs[b], 128).T)

    def pack_kv(x):
        # [512, cols] -> [128, NCH, 4, 128]
        ncols = x.shape[1]
        return np.ascontiguousarray(
            x.reshape(4, 128, ncols // 128, 128).transpose(1, 2, 0, 3))

    def pack_q(x):
        # [512, seq] -> [128, NT, 4, 512]
        ncols = x.shape[1]
        return np.ascontiguousarray(
            x.reshape(4, 128, ncols // 512, 512).transpose(1, 2, 0, 3))

    in_maps = []
    for core in range(N_CORES):
        hp, qh = divmod(core, 2)
        fs = hp * LOCAL_F
        wqkv = np.concatenate(
            [Wk[fs:fs + 128, :].T, Wq[fs:fs + 128, :].T,
             Wv[fs:fs + 128, :].T], axis=1)
        smalls = np.empty((128, 2 + LOCAL_F + NCH), np.float32)
        smalls[:, 0] = bq[fs:fs + 128]
        smalls[:, 1] = bk[fs:fs + 128]
        smalls[:, 2:2 + LOCAL_F] = bv[fs:fs + 128][None, :]
        smalls[:, 2 + LOCAL_F:2 + LOCAL_F + nch0] = masks[0]
        smalls[:, 2 + LOCAL_F + nch0:] = masks[1]
        qsl = slice(qh * QU, (qh + 1) * QU)
        xTq = np.concatenate(
            [queries[0, qsl].T, queries[1, qsl].T], axis=1)
        xTk = np.concatenate([keys[0, :sk0].T, keys[1, :sk1].T], axis=1)
        xTv = np.concatenate([values[0, :sk0].T, values[1, :sk1].T], axis=1)
        in_maps.append({
            "xq": pack_q(xTq).astype(npdt),
            "xk": pack_kv(xTk).astype(npdt),
            "xv": pack_kv(xTv).astype(npdt),
            "wqkv": np.ascontiguousarray(wqkv).astype(npdt),
            "woT": np.ascontiguousarray(Wo[:, fs:fs + 128].T).astype(npdt),
            "smalls": smalls,
        })
    return in_maps


# revision 16
# speedup vs baseline: 1.3037x; 1.2275x over previous
"""Trainium2 Bass kernel: masked multi-head attention (B=2, S=2048, D=512, H=8).

Sharding: (head-pair x query-half) across 8 cores (core = hp*2 + qh).
Every core processes BOTH batches for its 2 heads and its 1024 queries:
unit u0 = batch 0 (nch0 key chunks), unit u1 = batch 1 (nch1 chunks),
which balances work across cores when valid_lens differ per batch.

Inputs are packed chunk-major on the host so each projection can start
as soon as its own DMA lands: keys/values as [128, chunk, dc, 128],
queries as [128, qtile, dc, 512].  The attention inner loop interleaves
scores(c) / exp(c) / attn@v(c-1) so the in-order PE queue never blocks
on the ScalarE, keeping the tensor engine at full p-state.

Softmax: the mask is a per-partition bias on the exp activation; a ones
column in the augmented V gives the denominator as row 64 of the attn@v
PSUM tile.  Normalize does reciprocal straight off that PSUM row into
SBUF (one DVE op), broadcasts it across 64 partitions via a DRAM bounce
(DMA only — broadcast access patterns are legal on DRAM APs), then one
multiply per head reading the PSUM numerators directly.

The per-core partial out-projection [512, 1024] per batch is stored and
summed on the host (the "all-reduce"), then bias bo is added.

The kernel specializes on (nch0, nch1) = ceil(valid_len_b/128): key
positions >= valid_len get exp(-30000) == 0 weight, so chunks past the
bound are skipped.  Derived from runtime inputs -> correct for any
valid_lens.
"""

import math
import os
import sys

import numpy as np

for _p in ("/opt/trn_rl_repo",):
    if os.path.isdir(_p) and _p not in sys.path:
        sys.path.insert(0, _p)

import ml_dtypes

D_MODEL = 512
NUM_HEADS = 8
HEAD_DIM = 64
N_CORES = 8
LOCAL_F = 128            # features per core = 2 heads * 64
MASK_NEG = -30000.0

DT_NAME = os.environ.get("ATTN_KERNEL_DT", "bfloat16")
TRACE = False

last_results = None  # BassKernelResults of the most recent run (for test.py)

_PROG_CACHE = {}


def _np_dt(name):
    return ml_dtypes.bfloat16 if name == "bfloat16" else np.float32


def _build(nch0: int, nch1: int, seq: int, dt_name: str,
           qk_bias: bool, v_bias: bool):
    from contextlib import ExitStack

    import concourse.bass as bass  # noqa: F401
    import concourse.mybir as mybir
    import concourse.tile as tile
    from concourse import bacc

    DT = getattr(mybir.dt, dt_name)
    F32 = mybir.dt.float32
    EXP = mybir.ActivationFunctionType.Exp
    QU = seq // 2            # queries per unit
    assert QU % 512 == 0
    NQT = QU // 512          # 512-wide query tiles per unit
    NT = 2 * NQT             # query tiles total
    sk0, sk1 = nch0 * 128, nch1 * 128
    NCH = nch0 + nch1
    units = [(0, nch0, 0), (1, nch1, nch0)]  # (unit, nch, chunk offset)

    nc = bacc.Bacc("TRN2", target_bir_lowering=False, debug=False,
                   num_devices=N_CORES)

    def din(name, shape, dt=DT):
        return nc.dram_tensor(name, shape, dt, kind="ExternalInput").ap()

    # chunk-/tile-major packed inputs (see host packing in kernel())
    xq_d = din("xq", [128, NT, 4, 512])
    xk_d = din("xk", [128, NCH, 4, 128])
    xv_d = din("xv", [128, NCH, 4, 128])
    # [wk | wq | wv] column blocks, each 128 wide
    wqkv = din("wqkv", [D_MODEL, 3 * LOCAL_F])
    woT = din("woT", [LOCAL_F, D_MODEL])
    # f32 smalls: [bq | bk | bv | maskb(NCH)]
    NSM = 2 + LOCAL_F + NCH
    smalls_d = din("smalls", [128, NSM], F32)
    out_d = nc.dram_tensor("out", [D_MODEL, seq], DT,
                           kind="ExternalOutput").ap()

    with tile.TileContext(nc) as tc, ExitStack() as ctx:
        const = ctx.enter_context(tc.tile_pool(name="const", bufs=1))

        # ---- stage inputs into SBUF ----
        # Everything on the sync HWDGE queue in exact consumption order:
        # one queue's descriptors complete in order at full aggregate DMA
        # bandwidth, so the attention-critical prefix (wk+wq, k/q of unit
        # 0) lands first instead of time-sharing with later tensors.
        sm_sb = const.tile([128, NSM], F32, tag="sm")
        wqkv_sb = const.tile([128, 4, 3 * LOCAL_F], DT, tag="wqkv")
        wqkv_r = wqkv.rearrange("(c p) f -> p c f", p=128)
        xk_sb = const.tile([128, NCH, 4, 128], DT, tag="xk")
        xq_sb = const.tile([128, NT, 4, 512], DT, tag="xq")
        xv_sb = const.tile([128, NCH, 4, 128], DT, tag="xv")
        wo_sb = const.tile([LOCAL_F, D_MODEL], DT, tag="wo")
        nc.sync.dma_start(out=sm_sb, in_=smalls_d)
        nc.sync.dma_start(out=wqkv_sb[:, :, 0:2 * LOCAL_F],
                          in_=wqkv_r[:, :, 0:2 * LOCAL_F])
        nc.sync.dma_start(out=xk_sb[:, 0:nch0], in_=xk_d[:, 0:nch0])
        nc.sync.dma_start(out=xq_sb[:, 0], in_=xq_d[:, 0])
        nc.sync.dma_start(out=xq_sb[:, 1], in_=xq_d[:, 1])
        nc.sync.dma_start(out=xv_sb[:, 0:nch0], in_=xv_d[:, 0:nch0])
        nc.sync.dma_start(out=wqkv_sb[:, :, 2 * LOCAL_F:],
                          in_=wqkv_r[:, :, 2 * LOCAL_F:])
        nc.sync.dma_start(out=xk_sb[:, nch0:], in_=xk_d[:, nch0:])
        for t in range(2, NT):
            nc.sync.dma_start(out=xq_sb[:, t], in_=xq_d[:, t])
        for c in range(nch0, NCH):
            nc.sync.dma_start(out=xv_sb[:, c], in_=xv_d[:, c])
        nc.sync.dma_start(out=wo_sb, in_=woT)

        bq_sb = sm_sb[:, 0:1]
        bk_sb = sm_sb[:, 1:2]
        bv_sb = sm_sb[:, 2:2 + LOCAL_F]
        mb_sb = sm_sb[:, 2 + LOCAL_F:2 + LOCAL_F + NCH]
        wk_of, wq_of, wv_of = 0, LOCAL_F, 2 * LOCAL_F

        # ---- persistent SBUF operands ----
        qT = const.tile([LOCAL_F, seq], DT, tag="qT")
        kT = const.tile([LOCAL_F, NCH * 128], DT, tag="kT")
        # per chunk: [:, c, h, 0:64] = 1.0, [:, c, h, 64:128] = V_h — the
        # 64 ones columns replicate the softmax denominator into PSUM rows
        # 0..63 of the attn@v result (a free partition-broadcast: matmul
        # cost only depends on the moving size N, not on M), where the
        # custom reciprocal op can read it at base partition 0
        vaug = const.tile([128, NCH, 2, LOCAL_F], DT, tag="vaug")
        cn = const.tile([LOCAL_F, seq], DT, tag="cn")
        nc.vector.memset(vaug[:, :, :, 0:HEAD_DIM], 1.0)

        with (
            tc.tile_pool(name="swork", bufs=2, space="PSUM") as swork,
            tc.tile_pool(name="scps", bufs=2, space="PSUM") as scps,
            tc.tile_pool(name="otps", bufs=2, space="PSUM") as otps,
            tc.tile_pool(name="expp", bufs=7) as expp,
            tc.tile_pool(name="obp", bufs=4) as obp,
            tc.tile_pool(name="rbp", bufs=4) as rbp,
        ):
            # PE warm-up: ramps the HAM clock while the first DMAs land
            warm = const.tile([128, 512], DT, tag="warm")
            nc.vector.memset(warm, 0.0)
            for i in range(2):
                wps = swork.tile([128, 512], F32, tag="sw", name=f"warm{i}")
                nc.tensor.matmul(wps, lhsT=warm[:, 0:128], rhs=warm,
                                 start=True, stop=True)

            def k_proj_chunk(c, eng):
                kp = swork.tile([128, 512], F32, tag="sw", name=f"kp{c}")
                for dc in range(4):
                    nc.tensor.matmul(kp[:, 0:128],
                                     lhsT=wqkv_sb[:, dc, wk_of:wk_of + 128],
                                     rhs=xk_sb[:, c, dc, :],
                                     start=(dc == 0), stop=(dc == 3))
                dst = kT[:, c * 128:(c + 1) * 128]
                (nc.scalar.copy(out=dst, in_=kp[:, 0:128]) if eng == "s"
                 else nc.vector.tensor_copy(out=dst, in_=kp[:, 0:128]))
                if qk_bias:
                    nc.vector.tensor_scalar_add(out=dst, in0=dst,
                                                scalar1=bk_sb)

            def q_proj_tile(t, eng):
                qp = swork.tile([128, 512], F32, tag="sw", name=f"qp{t}")
                for dc in range(4):
                    nc.tensor.matmul(qp,
                                     lhsT=wqkv_sb[:, dc, wq_of:wq_of + 128],
                                     rhs=xq_sb[:, t, dc, :],
                                     start=(dc == 0), stop=(dc == 3))
                dst = qT[:, t * 512:(t + 1) * 512]
                (nc.scalar.copy(out=dst, in_=qp) if eng == "s"
                 else nc.vector.tensor_copy(out=dst, in_=qp))
                if qk_bias:
                    nc.vector.tensor_scalar_add(out=dst, in0=dst,
                                                scalar1=bq_sb)

            def v_proj_chunk(c, eng):
                vp = swork.tile([128, 512], F32, tag="sw", name=f"vp{c}")
                for dc in range(4):
                    nc.tensor.matmul(vp[:, 0:128],
                                     lhsT=xv_sb[:, c, dc, :],
                                     rhs=wqkv_sb[:, dc, wv_of:wv_of + 128],
                                     start=(dc == 0), stop=(dc == 3))
                if v_bias:
                    nc.vector.tensor_add(out=vp[:, 0:128], in0=vp[:, 0:128],
                                         in1=bv_sb)
                src = vp[:, 0:128].rearrange("p (h d) -> p h d", h=2)
                dst = vaug[:, c, :, HEAD_DIM:]
                (nc.scalar.copy(out=dst, in_=src) if eng == "s"
                 else nc.vector.tensor_copy(out=dst, in_=src))

            # projections for unit 0 first (attention starts on them);
            # everything later-staged drains inside the attention streams
            for c in range(nch0):
                k_proj_chunk(c, "s")
            q_proj_tile(0, "v")
            q_proj_tile(1, "v")

            def attention_head(u, h, pending, drain, lag=1):
                """scores/exp/attn@v for one (unit, head). `pending` PE work
                (late projections / out-proj of the previous unit) is
                drained `drain` items per chunk iteration, hiding it in the
                exp-wait gaps of the stream.  attn@v runs `lag` chunks
                behind scores so late-staged V chunks never stall the
                in-order PE queue."""
                _, nch, coff = units[u]
                ucol = u * QU
                exs = []
                oTs = {}

                def attn_v(c):
                    for t in range(NQT):
                        if c == 0:
                            oTs[t] = otps.tile([128, 512], F32, tag="oT",
                                               name=f"oT{u}{h}{t}")
                        nc.tensor.matmul(
                            oTs[t],
                            lhsT=vaug[:, coff + c, h, :],
                            rhs=exs[c][:, t * 512:(t + 1) * 512],
                            start=(c == 0), stop=(c == nch - 1))

                for c in range(nch):
                    sc = scps.tile([128, 1024], F32, tag="sc",
                                   name=f"sc{u}{h}{c}")
                    for t in range(NQT):
                        nc.tensor.matmul(
                            sc[:, t * 512:(t + 1) * 512],
                            lhsT=kT[h * 64:(h + 1) * 64,
                                    (coff + c) * 128:(coff + c + 1) * 128],
                            rhs=qT[h * 64:(h + 1) * 64,
                                   ucol + t * 512:ucol + (t + 1) * 512],
                            start=True, stop=True)
                    ex = expp.tile([128, 1024], DT, tag="ex",
                                   name=f"ex{u}{h}{c}")
                    nc.scalar.activation(
                        out=ex, in_=sc, func=EXP,
                        bias=mb_sb[:, coff + c:coff + c + 1],
                        scale=1.0 / math.sqrt(HEAD_DIM))
                    exs.append(ex)
                    if c >= lag:
                        attn_v(c - lag)
                    for _ in range(drain):
                        if pending:
                            item = pending.pop(0)
                            if item is not None:
                                item()
                while pending:
                    item = pending.pop(0)
                    if item is not None:
                        item()
                for c in range(max(0, nch - lag), nch):
                    attn_v(c)

                for t in range(NQT):
                    csl = slice(ucol + t * 512, ucol + (t + 1) * 512)
                    oT = oTs[t]
                    rb = rbp.tile([64, 512], F32, tag="rb",
                                  name=f"rb{u}{h}{t}")
                    nc.vector.reciprocal_approx_fast(
                        out=rb, in_=oT[0:64, :])
                    nc.vector.tensor_mul(
                        out=cn[h * 64:(h + 1) * 64, csl],
                        in0=oT[64:128, :], in1=rb)

            def out_proj_item(u, t, eng, odcs=(0, 1, 2, 3)):
                def emit():
                    csl = slice(u * QU + t * 512, u * QU + (t + 1) * 512)
                    for odc in odcs:
                        fp = swork.tile([128, 512], F32, tag="sw",
                                        name=f"fp{u}{t}{odc}")
                        nc.tensor.matmul(
                            fp, lhsT=wo_sb[:, odc * 128:(odc + 1) * 128],
                            rhs=cn[:, csl], start=True, stop=True)
                        ob = obp.tile([128, 512], DT, tag="ob",
                                      name=f"ob{u}{t}{odc}")
                        (nc.scalar.copy(out=ob, in_=fp) if eng == "s"
                         else nc.vector.tensor_copy(out=ob, in_=fp))
                        nc.gpsimd.dma_start(
                            out=out_d[odc * 128:(odc + 1) * 128, csl],
                            in_=ob)
                return emit

            # drain schedule matched to the DMA arrival order: unit-0 V
            # during u0h0; unit-1 k/q during u0h1; unit-1 V during u1h0
            # (attn@v lag 2 covers its late arrival); unit-0 out-proj
            # during u1h1 (its normalize is long done by then)
            p00 = [lambda c=c: v_proj_chunk(c, "v") for c in range(nch0)]
            attention_head(0, 0, p00, drain=1)
            p01 = [lambda c=c: k_proj_chunk(c, "v") for c in range(nch0, NCH)]
            p01 += [lambda: q_proj_tile(2, "v"), lambda: q_proj_tile(3, "v")]
            attention_head(0, 1, p01, drain=3)
            p10 = [None, None]
            p10 += [lambda c=c: v_proj_chunk(c, "v") for c in range(nch0, NCH)]
            attention_head(1, 0, p10, drain=2, lag=2)
            p11 = [out_proj_item(0, 0, "v", (0, 1)),
                   out_proj_item(0, 0, "v", (2, 3)),
                   out_proj_item(0, 1, "v", (0, 1)),
                   out_proj_item(0, 1, "v", (2, 3))]
            attention_head(1, 1, p11, drain=1)
            out_proj_item(1, 0, "s")()
            out_proj_item(1, 1, "v")()

    nc.compile()
    return nc


def kernel(queries, keys, values, valid_lens, Wq, bq, Wk, bk, Wv, bv, Wo, bo):
    global last_results
    queries = np.asarray(queries, dtype=np.float32)
    keys = np.asarray(keys, dtype=np.float32)
    values = np.asarray(values, dtype=np.float32)
    valid_lens = np.asarray(valid_lens).astype(np.int64)
    Wq = np.asarray(Wq, dtype=np.float32)
    Wk = np.asarray(Wk, dtype=np.float32)
    Wv = np.asarray(Wv, dtype=np.float32)
    Wo = np.asarray(Wo, dtype=np.float32)
    bq = np.asarray(bq, dtype=np.float32)
    bk = np.asarray(bk, dtype=np.float32)
    bv = np.asarray(bv, dtype=np.float32)
    bo = np.asarray(bo, dtype=np.float32)

    B, S, D = queries.shape
    assert (B, D) == (2, D_MODEL) and S % 2048 == 0
    QU = S // 2

    Ls = [int(min(max(int(valid_lens[b]), 1), S)) for b in range(B)]
    nchs = [(L + 127) // 128 for L in Ls]
    nch0, nch1 = nchs
    sk0, sk1 = nch0 * 128, nch1 * 128
    NCH = nch0 + nch1

    npdt = _np_dt(DT_NAME)
    qk_bias = bool(np.any(bq) or np.any(bk))
    v_bias = bool(np.any(bv))
    key = (nch0, nch1, S, DT_NAME, qk_bias, v_bias)
    if key not in _PROG_CACHE:
        _PROG_CACHE[key] = _build(nch0, nch1, S, DT_NAME, qk_bias, v_bias)
    nc = _PROG_CACHE[key]

    in_maps = _host_pack(queries, keys, values, nchs, Ls,
                         Wq, bq, Wk, bk, Wv, bv, Wo)

    from concourse.bass_utils import run_bass_kernel_spmd
    res = run_bass_kernel_spmd(nc, in_maps, list(range(N_CORES)), trace=TRACE)
    last_results = res
    outs = [r["out"] for r in res.results]

    final = np.empty((B, S, D), np.float32)
    for b in range(B):
        for qh in range(2):
            acc = sum(outs[hp * 2 + qh][:, b * QU:(b + 1) * QU]
                      .astype(np.float32) for hp in range(4))
            final[b, qh * QU:(qh + 1) * QU] = acc.T + bo
        if int(valid_lens[b]) == 0:
            # uniform attention over all S positions (reference semantics
            # when every key is masked: softmax of a constant row)
            row = (values[b].mean(0) @ Wv.T + bv) @ Wo.T + bo
            final[b] = np.broadcast_to(row, (S, D))
    return final


def _host_pack(queries, keys, values, nchs, Ls, Wq, bq, Wk, bk, Wv, bv, Wo):
    B, S, D = queries.shape
    QU = S // 2
    nch0, nch1 = nchs
    sk0, sk1 = nch0 * 128, nch1 * 128
    NCH = nch0 + nch1
    npdt = _np_dt(DT_NAME)

    masks = []
    for b in range(B):
        skb = nchs[b] * 128
        m = np.where(np.arange(skb) < Ls[b], 0.0, MASK_NEG).astype(np.float32)
        masks.append(m.reshape(nchs[b], 128).T)

    def pack_kv(x):
        # [512, cols] -> [128, NCH, 4, 128]
        ncols = x.shape[1]
        return np.ascontiguousarray(
            x.reshape(4, 128, ncols // 128, 128).transpose(1, 2, 0, 3))

    def pack_q(x):
        # [512, seq] -> [128, NT, 4, 512]
        ncols = x.shape[1]
        return np.ascontiguousarray(
            x.reshape(4, 128, ncols // 512, 512).transpose(1, 2, 0, 3))

    in_maps = []
    for core in range(N_CORES):
        hp, qh = divmod(core, 2)
        fs = hp * LOCAL_F
        wqkv = np.concatenate(
            [Wk[fs:fs + 128, :].T, Wq[fs:fs + 128, :].T,
             Wv[fs:fs + 128, :].T], axis=1)
        smalls = np.empty((128, 2 + LOCAL_F + NCH), np.float32)
        smalls[:, 0] = bq[fs:fs + 128]
        smalls[:, 1] = bk[fs:fs + 128]
        smalls[:, 2:2 + LOCAL_F] = bv[fs:fs + 128][None, :]
        smalls[:, 2 + LOCAL_F:2 + LOCAL_F + nch0] = masks[0]
        smalls[:, 2 + LOCAL_F + nch0:] = masks[1]
        qsl = slice(qh * QU, (qh + 1) * QU)
        xTq = np.concatenate(
            [queries[0, qsl].T, queries[1, qsl].T], axis=1)
        xTk = np.concatenate([keys[0, :sk0].T, keys[1, :sk1].T], axis=1)
        xTv = np.concatenate([values[0, :sk0].T, values[1, :sk1].T], axis=1)
        in_maps.append({
            "xq": pack_q(xTq).astype(npdt),
            "xk": pack_kv(xTk).astype(npdt),
            "xv": pack_kv(xTv).astype(npdt),
            "wqkv": np.ascontiguousarray(wqkv).astype(npdt),
            "woT": np.ascontiguousarray(Wo[:, fs:fs + 128].T).astype(npdt),
            "smalls": smalls,
        })
    return in_maps


# revision 19
# speedup vs baseline: 1.3622x; 1.0449x over previous
"""Trainium2 Bass kernel: masked multi-head attention (B=2, S=2048, D=512, H=8).

Sharding: (head-pair x query-half) across 8 cores (core = hp*2 + qh).
Every core processes BOTH batches for its 2 heads and its 1024 queries:
unit u0 = batch 0 (nch0 key chunks), unit u1 = batch 1 (nch1 chunks),
which balances work across cores when valid_lens differ per batch.

Inputs are packed chunk-major on the host so each projection can start
as soon as its own DMA lands: keys/values as [128, chunk, dc, 128],
queries as [128, qtile, dc, 512].  The attention inner loop interleaves
scores(c) / exp(c) / attn@v(c-1) so the in-order PE queue never blocks
on the ScalarE, keeping the tensor engine at full p-state.

Softmax: the mask is a per-partition bias on the exp activation; a ones
column in the augmented V gives the denominator as row 64 of the attn@v
PSUM tile.  Normalize does reciprocal straight off that PSUM row into
SBUF (one DVE op), broadcasts it across 64 partitions via a DRAM bounce
(DMA only — broadcast access patterns are legal on DRAM APs), then one
multiply per head reading the PSUM numerators directly.

The per-core partial out-projection [512, 1024] per batch is stored and
summed on the host (the "all-reduce"), then bias bo is added.

The kernel specializes on (nch0, nch1) = ceil(valid_len_b/128): key
positions >= valid_len get exp(-30000) == 0 weight, so chunks past the
bound are skipped.  Derived from runtime inputs -> correct for any
valid_lens.
"""

import math
import os
import sys

import numpy as np

for _p in ("/opt/trn_rl_repo",):
    if os.path.isdir(_p) and _p not in sys.path:
        sys.path.insert(0, _p)

import ml_dtypes

D_MODEL = 512
NUM_HEADS = 8
HEAD_DIM = 64
N_CORES = 8
LOCAL_F = 128            # features per core = 2 heads * 64
MASK_NEG = -30000.0

DT_NAME = os.environ.get("ATTN_KERNEL_DT", "bfloat16")
TRACE = False

last_results = None  # BassKernelResults of the most recent run (for test.py)

_PROG_CACHE = {}


def _np_dt(name):
    return ml_dtypes.bfloat16 if name == "bfloat16" else np.float32


def _inbuf_layout(nch0, nch1, seq):
    """Per-partition element offsets of the packed input buffer, in
    consumption order.  Returns (offsets dict, total width, dma groups)."""
    assert seq == 2048
    o = {}
    w = 0

    def add(name, n):
        nonlocal w
        o[name] = w
        w += n

    add("wkwq", 1024)           # [4dc, 256] wk|wq
    add("xk0", nch0 * 512)      # [nch0, 4dc, 128]
    add("xq0", 2048)            # [4dc, 512]
    add("xq1", 2048)
    add("xv0", nch0 * 512)
    add("wv", 512)              # [4dc, 128]
    add("xk1", nch1 * 512)
    add("xq2", 2048)
    add("xq3", 2048)
    add("xv1", nch1 * 512)
    add("wo", 512)
    groups = [
        (0, o["xq0"] + 2048),                  # wkwq + xk0 + xq_t0
        (o["xq1"], o["xq1"] + 2048),           # xq_t1
        (o["xv0"], o["xk1"]),                  # xv0 + wv
        (o["xk1"], o["xq3"]),                  # xk1 + xq_t2
        (o["xq3"], o["wo"]),                   # xq_t3 + xv1
        (o["wo"], w),                          # wo
    ]
    return o, w, groups


def _build(nch0: int, nch1: int, seq: int, dt_name: str,
           qk_bias: bool, v_bias: bool):
    from contextlib import ExitStack

    import concourse.bass as bass  # noqa: F401
    import concourse.mybir as mybir
    import concourse.tile as tile
    from concourse import bacc

    DT = getattr(mybir.dt, dt_name)
    F32 = mybir.dt.float32
    EXP = mybir.ActivationFunctionType.Exp
    QU = seq // 2            # queries per unit
    assert QU % 512 == 0
    NQT = QU // 512          # 512-wide query tiles per unit
    NT = 2 * NQT             # query tiles total
    sk0, sk1 = nch0 * 128, nch1 * 128
    NCH = nch0 + nch1
    units = [(0, nch0, 0), (1, nch1, nch0)]  # (unit, nch, chunk offset)

    nc = bacc.Bacc("TRN2", target_bir_lowering=False, debug=False,
                   num_devices=N_CORES)

    def din(name, shape, dt=DT):
        return nc.dram_tensor(name, shape, dt, kind="ExternalInput").ap()

    # all bf16 inputs packed per-partition-contiguously in consumption
    # order (see _inbuf_layout / host packing in kernel())
    LOF, LW, LGROUPS = _inbuf_layout(nch0, nch1, seq)
    inbuf_d = din("inbuf", [128, LW])
    # f32 smalls: [bq | bk | bv | maskb(NCH)]
    NSM = 2 + LOCAL_F + NCH
    smalls_d = din("smalls", [128, NSM], F32)
    out_d = nc.dram_tensor("out", [D_MODEL, seq], DT,
                           kind="ExternalOutput").ap()

    with tile.TileContext(nc) as tc, ExitStack() as ctx:
        const = ctx.enter_context(tc.tile_pool(name="const", bufs=1))

        # ---- stage inputs into SBUF ----
        # One packed buffer, DMA'd as consumption-ordered slices on the
        # sync HWDGE queue: a queue's descriptors complete in order at
        # full aggregate DMA bandwidth, and per-partition-contiguous
        # packing gives maximal descriptor sizes.
        sm_sb = const.tile([128, NSM], F32, tag="sm")
        nc.sync.dma_start(out=sm_sb, in_=smalls_d)
        inbuf = const.tile([128, LW], DT, tag="inbuf")
        for g0, g1 in LGROUPS:
            nc.sync.dma_start(out=inbuf[:, g0:g1], in_=inbuf_d[:, g0:g1])

        def view(name, n, *dims):
            ap = inbuf[:, LOF[name]:LOF[name] + n]
            if dims:
                pat = "p (" + " ".join(f"d{i}" for i in range(len(dims)))                     + ") -> p " + " ".join(f"d{i}" for i in range(len(dims)))
                ap = ap.rearrange(pat, **{f"d{i}": d
                                          for i, d in enumerate(dims)})
            return ap

        wkwq_sb = view("wkwq", 1024, 4, 256)     # [dc, wk|wq]
        wv_sb = view("wv", 512, 4, 128)
        xk_sbs = [view("xk0", nch0 * 512, nch0, 4, 128),
                  view("xk1", nch1 * 512, nch1, 4, 128)]
        xv_sbs = [view("xv0", nch0 * 512, nch0, 4, 128),
                  view("xv1", nch1 * 512, nch1, 4, 128)]
        xq_sbs = [view(f"xq{t}", 2048, 4, 512) for t in range(NT)]
        wo_sb = view("wo", 512)

        bq_sb = sm_sb[:, 0:1]
        bk_sb = sm_sb[:, 1:2]
        bv_sb = sm_sb[:, 2:2 + LOCAL_F]
        mb_sb = sm_sb[:, 2 + LOCAL_F:2 + LOCAL_F + NCH]

        # ---- persistent SBUF operands ----
        qT = const.tile([LOCAL_F, seq], DT, tag="qT")
        kT = const.tile([LOCAL_F, NCH * 128], DT, tag="kT")
        # per chunk: [:, c, h, 0:64] = 1.0, [:, c, h, 64:128] = V_h — the
        # 64 ones columns replicate the softmax denominator into PSUM rows
        # 0..63 of the attn@v result (a free partition-broadcast: matmul
        # cost only depends on the moving size N, not on M), where the
        # custom reciprocal op can read it at base partition 0
        vaug = const.tile([128, NCH, 2, LOCAL_F], DT, tag="vaug")
        cn = const.tile([LOCAL_F, seq], DT, tag="cn")
        nc.vector.memset(vaug[:, :, :, 0:HEAD_DIM], 1.0)

        with (
            tc.tile_pool(name="swork", bufs=2, space="PSUM") as swork,
            tc.tile_pool(name="scps", bufs=2, space="PSUM") as scps,
            tc.tile_pool(name="otps", bufs=2, space="PSUM") as otps,
            tc.tile_pool(name="expp", bufs=7) as expp,
            tc.tile_pool(name="obp", bufs=4) as obp,
            tc.tile_pool(name="rbp", bufs=4) as rbp,
        ):
            # PE warm-up: ramps the HAM clock while the first DMAs land
            warm = const.tile([128, 512], DT, tag="warm")
            nc.vector.memset(warm, 0.0)
            for i in range(2):
                wps = swork.tile([128, 512], F32, tag="sw", name=f"warm{i}")
                nc.tensor.matmul(wps, lhsT=warm[:, 0:128], rhs=warm,
                                 start=True, stop=True)

            def k_proj_chunk(c, eng):
                u, lc = (0, c) if c < nch0 else (1, c - nch0)
                kp = swork.tile([128, 512], F32, tag="sw", name=f"kp{c}")
                for dc in range(4):
                    nc.tensor.matmul(kp[:, 0:128],
                                     lhsT=wkwq_sb[:, dc, 0:128],
                                     rhs=xk_sbs[u][:, lc, dc, :],
                                     start=(dc == 0), stop=(dc == 3))
                dst = kT[:, c * 128:(c + 1) * 128]
                (nc.scalar.copy(out=dst, in_=kp[:, 0:128]) if eng == "s"
                 else nc.vector.tensor_copy(out=dst, in_=kp[:, 0:128]))
                if qk_bias:
                    nc.vector.tensor_scalar_add(out=dst, in0=dst,
                                                scalar1=bk_sb)

            def q_proj_tile(t, eng):
                qp = swork.tile([128, 512], F32, tag="sw", name=f"qp{t}")
                for dc in range(4):
                    nc.tensor.matmul(qp,
                                     lhsT=wkwq_sb[:, dc, 128:256],
                                     rhs=xq_sbs[t][:, dc, :],
                                     start=(dc == 0), stop=(dc == 3))
                dst = qT[:, t * 512:(t + 1) * 512]
                (nc.scalar.copy(out=dst, in_=qp) if eng == "s"
                 else nc.vector.tensor_copy(out=dst, in_=qp))
                if qk_bias:
                    nc.vector.tensor_scalar_add(out=dst, in0=dst,
                                                scalar1=bq_sb)

            def v_proj_chunk(c, eng):
                u, lc = (0, c) if c < nch0 else (1, c - nch0)
                vp = swork.tile([128, 512], F32, tag="sw", name=f"vp{c}")
                for dc in range(4):
                    nc.tensor.matmul(vp[:, 0:128],
                                     lhsT=xv_sbs[u][:, lc, dc, :],
                                     rhs=wv_sb[:, dc, :],
                                     start=(dc == 0), stop=(dc == 3))
                if v_bias:
                    nc.vector.tensor_add(out=vp[:, 0:128], in0=vp[:, 0:128],
                                         in1=bv_sb)
                src = vp[:, 0:128].rearrange("p (h d) -> p h d", h=2)
                dst = vaug[:, c, :, HEAD_DIM:]
                (nc.scalar.copy(out=dst, in_=src) if eng == "s"
                 else nc.vector.tensor_copy(out=dst, in_=src))

            # projections for unit 0 first (attention starts on them);
            # everything later-staged drains inside the attention streams
            for c in range(nch0):
                k_proj_chunk(c, "s")
            q_proj_tile(0, "v")
            q_proj_tile(1, "v")

            def attention_head(u, h, pending, drain, lag=1):
                """scores/exp/attn@v for one (unit, head). `pending` PE work
                (late projections / out-proj of the previous unit) is
                drained `drain` items per chunk iteration, hiding it in the
                exp-wait gaps of the stream.  attn@v runs `lag` chunks
                behind scores so late-staged V chunks never stall the
                in-order PE queue."""
                _, nch, coff = units[u]
                ucol = u * QU
                exs = []
                oTs = {}

                def attn_v(c):
                    for t in range(NQT):
                        if c == 0:
                            oTs[t] = otps.tile([128, 512], F32, tag="oT",
                                               name=f"oT{u}{h}{t}")
                        nc.tensor.matmul(
                            oTs[t],
                            lhsT=vaug[:, coff + c, h, :],
                            rhs=exs[c][:, t * 512:(t + 1) * 512],
                            start=(c == 0), stop=(c == nch - 1))

                for c in range(nch):
                    sc = scps.tile([128, 1024], F32, tag="sc",
                                   name=f"sc{u}{h}{c}")
                    for t in range(NQT):
                        nc.tensor.matmul(
                            sc[:, t * 512:(t + 1) * 512],
                            lhsT=kT[h * 64:(h + 1) * 64,
                                    (coff + c) * 128:(coff + c + 1) * 128],
                            rhs=qT[h * 64:(h + 1) * 64,
                                   ucol + t * 512:ucol + (t + 1) * 512],
                            start=True, stop=True)
                    ex = expp.tile([128, 1024], DT, tag="ex",
                                   name=f"ex{u}{h}{c}")
                    nc.scalar.activation(
                        out=ex, in_=sc, func=EXP,
                        bias=mb_sb[:, coff + c:coff + c + 1],
                        scale=1.0 / math.sqrt(HEAD_DIM))
                    exs.append(ex)
                    if c >= lag:
                        attn_v(c - lag)
                    for _ in range(drain):
                        if pending:
                            item = pending.pop(0)
                            if item is not None:
                                item()
                while pending:
                    item = pending.pop(0)
                    if item is not None:
                        item()
                for c in range(max(0, nch - lag), nch):
                    attn_v(c)

                for t in range(NQT):
                    csl = slice(ucol + t * 512, ucol + (t + 1) * 512)
                    oT = oTs[t]
                    rb = rbp.tile([64, 512], F32, tag="rb",
                                  name=f"rb{u}{h}{t}")
                    nc.vector.reciprocal_approx_fast(
                        out=rb, in_=oT[0:64, :])
                    nc.vector.tensor_mul(
                        out=cn[h * 64:(h + 1) * 64, csl],
                        in0=oT[64:128, :], in1=rb)

            def out_proj_item(u, t, eng, odcs=(0, 1, 2, 3)):
                def emit():
                    csl = slice(u * QU + t * 512, u * QU + (t + 1) * 512)
                    for odc in odcs:
                        fp = swork.tile([128, 512], F32, tag="sw",
                                        name=f"fp{u}{t}{odc}")
                        nc.tensor.matmul(
                            fp, lhsT=wo_sb[:, odc * 128:(odc + 1) * 128],
                            rhs=cn[:, csl], start=True, stop=True)
                        ob = obp.tile([128, 512], DT, tag="ob",
                                      name=f"ob{u}{t}{odc}")
                        e = eng if eng != "a" else ("s", "v")[odc % 2]
                        (nc.scalar.copy(out=ob, in_=fp) if e == "s"
                         else nc.vector.tensor_copy(out=ob, in_=fp))
                        nc.sync.dma_start(
                            out=out_d[odc * 128:(odc + 1) * 128, csl],
                            in_=ob)
                return emit

            # drain schedule matched to the DMA arrival order: unit-0 V
            # during u0h0; unit-1 k/q during u0h1; unit-1 V during u1h0
            # (attn@v lag 2 covers its late arrival); unit-0 out-proj
            # during u1h1 (its normalize is long done by then)
            p00 = [lambda c=c: v_proj_chunk(c, "v") for c in range(nch0)]
            attention_head(0, 0, p00, drain=1)
            p01 = [lambda c=c: k_proj_chunk(c, "v") for c in range(nch0, NCH)]
            p01 += [lambda: q_proj_tile(2, "v"), lambda: q_proj_tile(3, "v")]
            attention_head(0, 1, p01, drain=3)
            p10 = [None, None]
            p10 += [lambda c=c: v_proj_chunk(c, "v") for c in range(nch0, NCH)]
            p10 += [out_proj_item(0, 0, "v", (0, 1)),
                    out_proj_item(0, 0, "v", (2, 3))]
            attention_head(1, 0, p10, drain=2, lag=2)
            p11 = [out_proj_item(0, 1, "v", (0, 1)),
                   out_proj_item(0, 1, "v", (2, 3))]
            attention_head(1, 1, p11, drain=1)
            out_proj_item(1, 0, "a")()
            out_proj_item(1, 1, "a")()

    nc.compile()
    return nc


def kernel(queries, keys, values, valid_lens, Wq, bq, Wk, bk, Wv, bv, Wo, bo):
    global last_results
    queries = np.asarray(queries, dtype=np.float32)
    keys = np.asarray(keys, dtype=np.float32)
    values = np.asarray(values, dtype=np.float32)
    valid_lens = np.asarray(valid_lens).astype(np.int64)
    Wq = np.asarray(Wq, dtype=np.float32)
    Wk = np.asarray(Wk, dtype=np.float32)
    Wv = np.asarray(Wv, dtype=np.float32)
    Wo = np.asarray(Wo, dtype=np.float32)
    bq = np.asarray(bq, dtype=np.float32)
    bk = np.asarray(bk, dtype=np.float32)
    bv = np.asarray(bv, dtype=np.float32)
    bo = np.asarray(bo, dtype=np.float32)

    B, S, D = queries.shape
    assert (B, D) == (2, D_MODEL) and S == 2048
    QU = S // 2

    Ls = [int(min(max(int(valid_lens[b]), 1), S)) for b in range(B)]
    nchs = [(L + 127) // 128 for L in Ls]
    nch0, nch1 = nchs
    sk0, sk1 = nch0 * 128, nch1 * 128
    NCH = nch0 + nch1

    npdt = _np_dt(DT_NAME)
    qk_bias = bool(np.any(bq) or np.any(bk))
    v_bias = bool(np.any(bv))
    key = (nch0, nch1, S, DT_NAME, qk_bias, v_bias)
    if key not in _PROG_CACHE:
        _PROG_CACHE[key] = _build(nch0, nch1, S, DT_NAME, qk_bias, v_bias)
    nc = _PROG_CACHE[key]

    in_maps = _host_pack(queries, keys, values, nchs, Ls,
                         Wq, bq, Wk, bk, Wv, bv, Wo)

    from concourse.bass_utils import run_bass_kernel_spmd
    res = run_bass_kernel_spmd(nc, in_maps, list(range(N_CORES)), trace=TRACE)
    last_results = res
    outs = [r["out"] for r in res.results]

    final = np.empty((B, S, D), np.float32)
    for b in range(B):
        for qh in range(2):
            acc = sum(outs[hp * 2 + qh][:, b * QU:(b + 1) * QU]
                      .astype(np.float32) for hp in range(4))
            final[b, qh * QU:(qh + 1) * QU] = acc.T + bo
        if int(valid_lens[b]) == 0:
            # uniform attention over all S positions (reference semantics
            # when every key is masked: softmax of a constant row)
            row = (values[b].mean(0) @ Wv.T + bv) @ Wo.T + bo
            final[b] = np.broadcast_to(row, (S, D))
    return final


def _host_pack(queries, keys, values, nchs, Ls, Wq, bq, Wk, bk, Wv, bv, Wo):
    B, S, D = queries.shape
    QU = S // 2
    nch0, nch1 = nchs
    sk0, sk1 = nch0 * 128, nch1 * 128
    NCH = nch0 + nch1
    npdt = _np_dt(DT_NAME)
    LOF, LW, _ = _inbuf_layout(nch0, nch1, S)

    masks = []
    for b in range(B):
        skb = nchs[b] * 128
        m = np.where(np.arange(skb) < Ls[b], 0.0, MASK_NEG).astype(np.float32)
        masks.append(m.reshape(nchs[b], 128).T)

    def pack_kv(x):
        # [512, cols] -> [128, cols/128, 4, 128] flattened per partition
        ncols = x.shape[1]
        return x.reshape(4, 128, ncols // 128, 128).transpose(1, 2, 0, 3)                 .reshape(128, -1)

    def pack_q(x):
        # [512, 512] one q tile -> [128, 4, 512] flattened
        return x.reshape(4, 128, 512).transpose(1, 0, 2).reshape(128, -1)

    def pack_w(w):
        # [512, ncols] -> [128, 4, ncols] flattened
        ncols = w.shape[1]
        return w.reshape(4, 128, ncols).transpose(1, 0, 2).reshape(128, -1)

    in_maps = []
    for core in range(N_CORES):
        hp, qh = divmod(core, 2)
        fs = hp * LOCAL_F
        wkwq = np.concatenate(
            [Wk[fs:fs + 128, :].T, Wq[fs:fs + 128, :].T], axis=1)
        smalls = np.empty((128, 2 + LOCAL_F + NCH), np.float32)
        smalls[:, 0] = bq[fs:fs + 128]
        smalls[:, 1] = bk[fs:fs + 128]
        smalls[:, 2:2 + LOCAL_F] = bv[fs:fs + 128][None, :]
        smalls[:, 2 + LOCAL_F:2 + LOCAL_F + nch0] = masks[0]
        smalls[:, 2 + LOCAL_F + nch0:] = masks[1]
        qsl = slice(qh * QU, (qh + 1) * QU)
        xTq = [np.ascontiguousarray(queries[b, qsl].T) for b in range(2)]
        parts = [
            pack_w(wkwq),
            pack_kv(keys[0, :sk0].T),
            pack_q(xTq[0][:, 0:512]),
            pack_q(xTq[0][:, 512:1024]),
            pack_kv(values[0, :sk0].T),
            pack_w(Wv[fs:fs + 128, :].T),
            pack_kv(keys[1, :sk1].T),
            pack_q(xTq[1][:, 0:512]),
            pack_q(xTq[1][:, 512:1024]),
            pack_kv(values[1, :sk1].T),
            np.ascontiguousarray(Wo[:, fs:fs + 128].T),
        ]
        inbuf = np.concatenate(parts, axis=1)
        assert inbuf.shape == (128, LW), (inbuf.shape, LW)
        in_maps.append({
            "inbuf": np.ascontiguousarray(inbuf).astype(npdt),
            "smalls": smalls,
        })
    return in_maps
